# revision 62
# baseline (speedup 1.0000x reference)
"""Trainium2 Bass kernel for nn_AttentionMemory (sparse_attention).

Reference computation (per batch b):
    mk = Mk[b].reshape(CK, N); qk = Qk[b].reshape(CK, N)
    affinity[m, q] = softmax_m( (2*mk[:,m]@qk[:,q] - |mk[:,m]|^2) / sqrt(CK) )

Sharding: 8 cores = 4 batches x 2 query-halves. Each core computes the full
memory (softmax) axis for 2048 of one batch's queries — no collectives.

Per-core layout: queries on partitions (16 q-tiles of 128), memory positions
on the free axis; softmax runs along the free axis so every q-tile completes
independently and output DMA streams from the start.  ScalarE's exp is the
hard floor: 8.4M elements/core at ~0.96 ns/element = 63 us busy; everything
else is arranged to keep ScalarE saturated (measured per-tile period
4343 ns = ACT 1966 x2 + one 288 ns accumulator read + ~100 ns gaps):

- The a_sq term: DVE squares mk chunk-by-chunk as it lands (idle during
  startup, 2x mode); contracting mksq = mk^2 with an all(-0.5) [128,128]
  stationary yields -0.5*a_sq in PSUM (start=True), and the ab matmuls
  accumulate on top.  Keeping inputs to mk+qk only (1.5MB) makes startup
  input-DMA-bound at ~14.5us all-in.  (A K=1 ones-row broadcast of a
  host-side a_sq row costs the same PE streaming time but moves 1/128th
  the data — and the HAM clock governor DEMOTES the PE to 1.2 GHz when its
  epoch data activity is too low, which cost 35% PE clock for the run.)
- Per q-tile the 4096-wide row is built in two [128, 2048] PSUM tiles
  (4 banks each, ping-pong): 4 mksq matmuls (start) + 4 ab matmuls (stop),
  512 cols each (1024-wide fp16 fails the s3d3 ISA check).
- ScalarE: one 2048-wide exp per PSUM tile.  Only half A carries accum_out
  (row-sum): the ACTIVATION_READ_ACCUMULATOR aux op costs ~288 ns of
  serialized ScalarE time, so half B's row-sum runs on DVE instead
  (tensor_reduce, 1x mode, ~2.3 us — DVE has the slack; ScalarE does not).
  The Exp table is preloaded by a dummy 1-wide ACT during startup (saves a
  1.5 us mid-pipeline ACT_TABLE_LOAD).  Logits are bounded (~[-30, +8]) so
  no max-subtraction pass is needed.
- DVE: sumB + add + reciprocal, then two in-place [128, 2048] bf16
  tensor_scalar multiplies normalize exp_t; each half DMAs out as soon as
  it is scaled.  The host casts bf16->fp32 and transposes while gathering
  (bf16 probabilities add ~0.4% error; budget is 2e-2).  DVE is near
  critical (~4.1 us/tile); no cheaper free-axis reduce exists (tensor_
  reduce/pool/bn_stats all lack DVE 2x modes, tensor_scalar+accum drops to
  1x, GpSimd reduces are cross-partition only).
- Output DMA: A-halves on the SP HW queue, B-halves on the GpSimd SW queue
  (~120 GB/s, high latency — fine mid-run, so the last two tiles stay on
  SP and the final tile splits 4-ways across SP + the then-idle Activation
  queue to shorten the post-ACT tail).
- Input DMA: mksq+mk interleave across the SP and Activation HW queues in
  consumption order (~256KB per 1.2 us per queue after a ~4.5 us latency).
- HAM management: the clock governor evaluates ~3.4 us epochs; a PE idle
  gap of ~1 us (or a low-data-activity epoch) drops 2.4 -> 1.2 GHz, and
  recovery needs a busy epoch.  N_WARM warmup matmuls cover the DMA wait
  and PAD_T0 pad matmuls (anchored by PSUM WAW deps; the following asq
  group's start=True overwrites them) absorb arrival jitter.  Dependency-
  free pads get hoisted by the Tile scheduler and LDWEIGHTS-only activity
  does not count — pads must be real matmuls.

Walrus caps instructions at one sync wait; _strip_self_waits spills extra
waits onto single-wait Drain instructions (semantically equivalent — waits
are an AND over monotonic semaphores, executed in order on one sequencer).
"""
import math
import numpy as np

import bass_rust
from concourse import bass, tile, mybir
from concourse.alu_op_type import AluOpType
from concourse.bass_utils import run_bass_kernel_spmd

B, CK, HH, WW = 4, 128, 64, 64
N = HH * WW            # 4096 memory positions / queries per batch
QH = N // 2            # 2048 queries per core
N_CORES = 8
QTILE = 128            # queries per q-tile (PSUM partition dim)
MCHUNK = 512           # memory cols per matmul (one PSUM bank of fp32)
MHALF = 2048           # memory cols per PSUM tile / exp instruction
SCALE = 2.0 / math.sqrt(CK)
N_WARM = 48            # HAM warmup matmuls: the PE clock ramp (1.2->2.4 GHz)
                       # needs ~4us of SUSTAINED activity (~107ns each), and
                       # any later PE idle gap of ~1us drops it back with no
                       # recovery (steady state never has 3.4us of continuous
                       # busy, so the whole run would stay at 1.2 GHz: +35us).
                       # The warmup must also END no earlier than the first
                       # real matmul's operands land (asq + mk0, ~13.3us).
# Pad matmuls before each asq group, written into the same PSUM tile the
# asq group then overwrites with start=True (harmless, and the WAW dep
# anchors them in schedule order — dependency-free pads get hoisted to the
# start by the Tile scheduler, and LDWEIGHTS-only activity does not count
# for the HAM).  Two jobs:
#  - startup (t0/t1): absorb mk DMA arrival jitter so no >1us PE idle gap;
#  - steady state: the HAM re-evaluates the PE clock every ~3.4us epoch and
#    demotes 2.4->1.2 GHz if epoch utilization drops below ~75-80% (observed:
#    72% util died after exactly 3 epochs; baseline's 80% survived).  Pads
#    lift PE util to ~90% and fit entirely in PE slack (ScalarE paces).
PAD_T0 = {(0, 1): 4, (1, 0): 4, (1, 1): 2, (2, 0): 2}
PAD_STEADY = 0


def _pad_count(t, h):
    return PAD_T0.get((t, h), PAD_STEADY)
F32 = mybir.dt.float32
F16 = mybir.dt.float16
BF16 = mybir.dt.bfloat16


def _build():
    nc = bass.Bass("TRN2", target_bir_lowering=False, debug=False,
                   num_devices=N_CORES)
    mk_d = nc.dram_tensor("mk", [CK, N], F16, kind="ExternalInput").ap()
    qk_d = nc.dram_tensor("qk", [CK, QH], F16, kind="ExternalInput").ap()
    out_d = nc.dram_tensor("out", [QH, N], BF16, kind="ExternalOutput").ap()

    n_qt = QH // QTILE          # 16
    with tile.TileContext(nc) as tc:
        with tc.tile_pool(name="inp", bufs=1) as inp_pool, \
             tc.tile_pool(name="exp", bufs=4) as exp_pool, \
             tc.tile_pool(name="small", bufs=8) as small_pool, \
             tc.tile_pool(name="psum", bufs=2, space="PSUM") as psum_pool:

            mk_sb = inp_pool.tile([CK, N], F16, tag="mk")
            qk_sb = inp_pool.tile([CK, QH], F16, tag="qk")
            msq_sb = inp_pool.tile([CK, N], F16, tag="mksq")
            # All -0.5 stationary: contracts the on-device mksq = mk^2 into
            # -0.5*a_sq per memory column.  Also the warmup operand.
            warm = inp_pool.tile([128, QTILE], F16, tag="warm")
            biasc = inp_pool.tile([QTILE, 1], F32, tag="biasc")
            nc.vector.memset(warm[:], -0.5)
            nc.vector.memset(biasc[:], 0.0)

            # Each HW-DGE queue streams ~256KB per 1.2us after a ~4.5us
            # latency (first item lands ~13us, +-0.5us jitter).  Only mk+qk
            # stream in (1.5MB over two queues, all-in ~14.5us); mksq = mk^2
            # is squared on-device by DVE (idle during startup, 2x mode)
            # chunk-by-chunk as mk lands.
            # mk's last chunk gates t0.B's whole chain (TT4 -> asq_B ->
            # ACT_B), so it takes the SP queue's 3rd slot (~14.6us) instead
            # of the Activation queue's (measured 15.3us); qk_rest is not
            # needed until t1's stationary load (~18us) and swaps places.
            for sb, dr, lo, hi, eng in (
                    (mk_sb, mk_d, 0, 1024, nc.sync),
                    (mk_sb, mk_d, 1024, 2048, nc.scalar),
                    (qk_sb, qk_d, 0, QTILE, nc.scalar),
                    (mk_sb, mk_d, 2048, 3072, nc.sync),
                    (mk_sb, mk_d, 3072, 3584, nc.sync),
                    (mk_sb, mk_d, 3584, 4096, nc.scalar),
                    (qk_sb, qk_d, QTILE, QH, nc.scalar)):
                eng.dma_start(out=sb[:, lo:hi], in_=dr[:, lo:hi])
            # 512-wide pieces: each asq matmul chunk unblocks on a half-size
            # square, pulling the fill-phase ACT chain ~0.2-0.3us earlier.
            for c in range(8):
                nc.vector.tensor_mul(msq_sb[:, c * 512:(c + 1) * 512],
                                     mk_sb[:, c * 512:(c + 1) * 512],
                                     mk_sb[:, c * 512:(c + 1) * 512])

            # Preload the Exp activation table (1.5us) during startup so the
            # first real ACT doesn't pay for it mid-pipeline.
            dummy = small_pool.tile([QTILE, 1], BF16, tag="dummy")
            nc.scalar.activation(dummy[:], biasc[:],
                                 mybir.ActivationFunctionType.Exp,
                                 scale=SCALE, bias=biasc[:])

            # Warm the PE's HAM clock gate with throwaway matmuls that only
            # need the memset warm tile, overlapping the input-DMA wait.
            warm_ps = psum_pool.tile([QTILE, MHALF], F32, tag="ps")
            for w in range(N_WARM):
                nc.tensor.matmul(warm_ps[:, 0:QTILE], warm[:], warm[:],
                                 start=True, stop=True)

            deferred = []       # (sbuf slice, dram slice) DMAs issued late
            for t in range(n_qt):
                qk_t = qk_sb[:, t * QTILE:(t + 1) * QTILE]
                exp_t = exp_pool.tile([QTILE, N], BF16, tag="exp")
                parts = small_pool.tile([QTILE, 2], F32, tag="parts")
                s_t = small_pool.tile([QTILE, 1], F32, tag="S")
                rec_t = small_pool.tile([QTILE, 1], F32, tag="rec")
                last = t == n_qt - 1
                for h in range(2):
                    ps = psum_pool.tile([QTILE, MHALF], F32, tag="ps")
                    for _ in range(_pad_count(t, h)):
                        nc.tensor.matmul(ps[:, 0:QTILE], warm[:], warm[:],
                                         start=True, stop=True)
                    for c in range(4):
                        m0 = h * MHALF + c * MCHUNK
                        nc.tensor.matmul(ps[:, c * MCHUNK:(c + 1) * MCHUNK],
                                         warm[:], msq_sb[:, m0:m0 + MCHUNK],
                                         start=True, stop=False)
                    for c in range(4):
                        m0 = h * MHALF + c * MCHUNK
                        nc.tensor.matmul(ps[:, c * MCHUNK:(c + 1) * MCHUNK],
                                         qk_t, mk_sb[:, m0:m0 + MCHUNK],
                                         start=False, stop=True)
                    accum = parts[:, h:h + 1] if (h == 0 or last) else None
                    nc.scalar.activation(
                        exp_t[:, h * MHALF:(h + 1) * MHALF], ps[:],
                        mybir.ActivationFunctionType.Exp, scale=SCALE,
                        bias=biasc[:], accum_out=accum)
                # Denominator: half A from ScalarE's accumulator, half B on
                # DVE (keeps a 340ns ACTIVATION_READ_ACCUMULATOR off the
                # saturated ScalarE).  Last tile: both halves from ScalarE so
                # the post-ACT tail skips the 2.2us DVE reduce.
                if last:
                    nc.vector.tensor_add(s_t[:], parts[:, 0:1], parts[:, 1:2])
                else:
                    sumb = small_pool.tile([QTILE, 1], F32, tag="sumb")
                    nc.vector.tensor_reduce(sumb[:], exp_t[:, MHALF:N],
                                            mybir.AxisListType.X,
                                            AluOpType.add)
                    nc.vector.tensor_add(s_t[:], parts[:, 0:1], sumb[:])
                nc.vector.reciprocal(rec_t[:], s_t[:])
                # Normalize in place + store per half: output DMA of half h
                # starts while half h+1 is still being scaled.  A-halves go
                # on the SP queue, B-halves on the (otherwise idle) GpSimd
                # queue so neither queue saturates.  The last tile is split
                # into 4 pipelined chunks across both queues to shorten the
                # post-ACT tail.
                if last:
                    # Flush deferred DMAs first: on the Activation SEQ they
                    # must precede t15's normalize-gated issues (in-order
                    # SEQ; a blocked wait would strand them), and by now the
                    # last ACT has been dispatched so they cost ScalarE
                    # nothing.
                    for sb_ap, dr_ap in deferred:
                        nc.scalar.dma_start(out=dr_ap, in_=sb_ap)
                    # ScalarE is done: its HW queue helps drain the tail.
                    chunks = ((0, 1024, nc.sync), (1024, MHALF, nc.scalar),
                              (MHALF, 3072, nc.sync), (3072, N, nc.scalar))
                elif t < n_qt - 2:
                    chunks = ((0, MHALF, nc.sync), (MHALF, N, nc.gpsimd))
                else:
                    # GpSimd's software queue (~120 GB/s, high latency) must
                    # not carry the final tiles, and t14's B-half would
                    # congest the SP queue right when t15's chunks need it —
                    # defer it to the Activation queue (issued above, once
                    # ScalarE's ACT chain is done).
                    chunks = ((0, MHALF, nc.sync), (MHALF, N, None))
                for lo, hi, eng in chunks:
                    nc.vector.tensor_scalar_mul(
                        exp_t[:, lo:hi], exp_t[:, lo:hi], rec_t[:])
                    dst = out_d[t * QTILE:(t + 1) * QTILE, lo:hi]
                    if eng is None:
                        deferred.append((exp_t[:, lo:hi], dst))
                    else:
                        eng.dma_start(out=dst, in_=exp_t[:, lo:hi])
    _strip_self_waits(nc)
    return nc


def _strip_self_waits(nc):
    """Walrus rejects instructions carrying more than one sync wait.

    Conservative fix: for any instruction with N>1 waits, keep the last wait
    on the instruction and spill the other N-1 onto single-wait Drain
    instructions inserted immediately before it on the same engine. All waits
    still execute, in program order, on the same sequencer; semaphores are
    monotonic so splitting an AND of waits into a sequence is equivalent.
    """
    for fn in nc.m.functions:
        for blk in fn.blocks:
            il = blk.instructions
            new_il = []
            changed = False
            for ins in il:
                si = getattr(ins, "sync_info", None)
                if si is not None and len(si.on_wait) > 1:
                    changed = True
                    waits = list(si.on_wait)
                    for k, w in enumerate(waits[:-1]):
                        d = mybir.InstDrain(
                            name=f"{ins.name}_w{k}",
                            ins=[], outs=[], bass_is_fusable=False)
                        d.engine = ins.engine
                        d.sync_info = bass_rust.SyncInfo(on_wait=[w],
                                                         on_update=[])
                        new_il.append(d)
                    ins.sync_info = bass_rust.SyncInfo(on_wait=[waits[-1]],
                                                      on_update=si.on_update)
                new_il.append(ins)
            if changed:
                blk.instructions = new_il


_NC_CACHE = None


def _make_in_maps(Mk: np.ndarray, Qk: np.ndarray) -> list[dict]:
    Mk = np.ascontiguousarray(np.asarray(Mk), dtype=np.float32)
    Qk = np.ascontiguousarray(np.asarray(Qk), dtype=np.float32)
    in_maps = []
    mks = {b: np.ascontiguousarray(Mk[b].reshape(CK, N).astype(np.float16))
           for b in range(B)}
    for c in range(N_CORES):
        b, half = c // 2, c % 2
        qk = np.ascontiguousarray(
            Qk[b].reshape(CK, N)[:, half * QH:(half + 1) * QH]
            .astype(np.float16))
        in_maps.append({"mk": mks[b], "qk": qk})
    return in_maps


def kernel(Mk: np.ndarray, Qk: np.ndarray) -> np.ndarray:
    global _NC_CACHE
    if _NC_CACHE is None:
        _NC_CACHE = _build()
    nc = _NC_CACHE

    in_maps = _make_in_maps(Mk, Qk)

    res = run_bass_kernel_spmd(nc, in_maps, core_ids=list(range(N_CORES)))

    out = np.empty((B, N, N), dtype=np.float32)
    for c in range(N_CORES):
        b, half = c // 2, c % 2
        out[b, :, half * QH:(half + 1) * QH] = \
            res.results[c]["out"].astype(np.float32).T
    return out
